# revision 61
# baseline (speedup 1.0000x reference)
"""Dilated MHSA block on 8 Trainium2 NeuronCores.

Sharding: sequence-parallel. Core c (0..7) handles batch b=c//4, query chunk
[512*(c%4), 512*(c%4)+512) with a 16-token halo of keys/values on each side.
Each core computes its full 512x1024 output slice; the host just concatenates.

All heavy matmuls take fp16 inputs (1 cycle/row on the PE at any moving dim;
an fp16 input carries the same 11-bit significand the PE's tf32/fp32r mode
would keep from fp32) and accumulate in fp32 PSUM, so inputs ship as fp16 -
half the DMA traffic - at fp32r-equivalent accuracy (~5e-4 rel err).
Normalization scalars stay fp32/fp32r.

Per-core device pipeline:
  1a. q,k projection qkT = Wqk @ x^T in (feature, token) layout, one weight
      DMA per feature-tile pair; ACT evicts PSUM with per-feature bias
      (Identity) and squares (Square); per-pair norm reductions via selector
      matmuls are software-pipelined one tile behind the main matmuls.
  1b. QK-norm: ACT sqrt, DVE eps+reciprocal; q is normalized via a rank-2
      broadcast matmul + DVE multiply; k's factor is folded into the exp
      scale (per-partition in keys-on-partitions layout) after PE-transposing
      the (16, 544) factor block to (token, head) tiles.
  1c. v projection in natural (token, feat) layout with token tiles shifted
      -16 so banded attention reads aligned v tiles; heads stored at stride
      65 with a ones column so the AV matmul also emits the softmax
      denominator. Overlapped with the j=0 attention sweep (PE-heavy vs
      DVE/ACT-heavy) under a shared PSUM budget.
  2.  Per (head, 256-query block): 3 score matmuls restricted to each key
      chunk's live query window (keys 128/128/32 on partitions, 160/160/32
      q free), one merged DVE mask-add over the two live 160-wide windows
      (strided 3D AP) + a (32, 32) add for the tail chunk, ACT exp on live
      windows only (scale = 1/(|k|+eps)), 3 AV matmuls; adjacent head pairs
      pack their AV outputs into one (65, 512) PSUM bank so the DVE
      reciprocal, rank-1 broadcast matmul, and ACT evict run once per pair
      at 512 width, then per-head DVE multiplies -> normalized outT.
      P-tile dead regions are zeroed once via DMA and never rewritten.
  3.  Output projection y = outT^T @ out_w^T + b (bias as rank-1 K-append),
      interleaved per query block with the attention sweeps so the PE-bound
      projection hides under elementwise-bound attention.
"""

import os
import sys

for _p in ("/opt/trn_rl_repo", "/root/.axon_site/_ro/trn_rl_repo"):
    if os.path.isdir(_p) and _p not in sys.path:
        sys.path.insert(0, _p)

import ml_dtypes
import numpy as np

import concourse.bass as bass
import concourse.mybir as mybir
import concourse.tile as tile
from concourse import bacc
from concourse import bass_utils

F32 = mybir.dt.float32
FR = mybir.dt.float32r  # fp32 bits, tf32 matmul mode (full rate at N >= 256)
F16 = mybir.dt.float16
F8 = mybir.dt.float8e4  # e4m3; q/k projection runs fp8 DoubleRow (2x PE)

B, N, D = 2, 2048, 1024
H, DH = 16, 64
KWIN, DIL = 8, 2
EPS = 1e-6
NCORES = 8
CHUNK = 512          # queries per core
HALO = 16            # KWIN * DIL
LOCAL = CHUNK + 2 * HALO  # 544 tokens (keys/values) per core
NEG = -30000.0       # mask value (exp(NEG * scale) == 0 for any sane scale)
KT = D // 128        # 8 contraction tiles


def _win(ap2d, stride, count, width):
    """(P, count, width) windowed view over a 2D AP's free dim."""
    return bass.AP(
        tensor=ap2d.tensor,
        offset=ap2d.offset,
        ap=[list(ap2d.ap[0]), [stride, count], [1, width]],
    )


def _win4(ap2d, s2, s1, width):
    """(P, 2, 2, width) doubly-windowed view over a 2D AP's free dim."""
    return bass.AP(
        tensor=ap2d.tensor,
        offset=ap2d.offset,
        ap=[list(ap2d.ap[0]), [s2, 2], [s1, 2], [1, width]],
    )


def _emit(tc, T):
    nc = tc.nc
    AF = mybir.ActivationFunctionType
    OP = mybir.AluOpType

    with tc.tile_pool(name="persist", bufs=1) as pp:
        # ---- early loads -------------------------------------------------
        xT = pp.tile([128, KT, LOCAL], F16)          # x^T, (in-feat, token)
        for kt in range(KT):
            nc.sync.dma_start(xT[:, kt], T["xT"][128 * kt : 128 * (kt + 1)])
        xT8 = pp.tile([128, KT, LOCAL], F8)          # fp8 copy for q/k proj
        nc.sync.dma_start(xT8, T["xT8"].rearrange("(kt p) n -> p kt n", p=128))

        wq_prefetch = {}

        qb2 = pp.tile([128, 16], F32)                # qkv bias for q,k
        sel = pp.tile([128, 2], F16)                  # head-pair sum selector
        sel16 = pp.tile([128, 8, 16], F16)            # k-side scatter selectors
        selT = pp.tile([2, 128], F16)                # head-pair bcast selector
        selT16 = pp.tile([16, 8, 128], F16)          # k-pair bcast selectors
        ones1 = pp.tile([1, 128], F16)
        ones1r = pp.tile([1, 64], F16)
        eps2 = pp.tile([2, 1], F32)
        nc.gpsimd.memset(eps2, 1e-8)
        eps16 = pp.tile([16, 1], F32)
        nc.gpsimd.memset(eps16, 1e-8)
        rk = pp.tile([16, LOCAL], F16)               # 1/|k| per (head, token)

        v_sb = pp.tile([128, 5, H * 65], F16)         # v natural, 65-stride heads
        v_h = v_sb.rearrange("p m (h c) -> p m h c", c=65)

        pabs = [pp.tile([128, 1024], F16, name=f"pab{i}") for i in range(3)]
        pcs = [pp.tile([32, 512], F16, name=f"pc{i}") for i in range(3)]
        masks = pp.tile([128, 2, 2, 3, 256], F16)    # (p, j, rep, chunk, q)
        vb = pp.tile([1, D], F16)
        ob = pp.tile([1, D], F16)

        k_sb = pp.tile([128, 8, LOCAL], F16)          # k^T normalized, 2 heads/tile
        qn_sb = pp.tile([128, 8, LOCAL], F16)         # q^T normalized
        outTn = pp.tile([128, KT, CHUNK], F16)        # attn out^T (feat, q)

        # ---- phase 1a/1b: q,k projection + QK norm ----------------------
        _do_p1 = int(os.environ.get("PROBE_ABLATE", "0")) < 4
        with (
            tc.tile_pool(name="wpool", bufs=8) as wpool,
            tc.tile_pool(name="sqpool", bufs=6) as sqpool,
            tc.tile_pool(name="ps1", bufs=2, space="PSUM") as ps1,
            tc.tile_pool(name="bcp", bufs=2, space="PSUM") as bcp,
        ):
            for mp in range(2):
                wt = wpool.tile([128, KT, 256], F8, tag="wqk", name=f"wtp{mp}")
                nc.sync.dma_start(
                    wt,
                    T["wqk"][:, 256 * mp : 256 * (mp + 1)].rearrange(
                        "(kt p) n -> p kt n", p=128
                    ),
                )
                wq_prefetch[mp] = wt
            nc.sync.dma_start(qb2, T["qb2"].rearrange("(m p) -> p m", p=128))
            nc.sync.dma_start(sel, T["sel"])
            nc.sync.dma_start(sel16, T["sel16"].rearrange("g p c -> p g c"))
            nc.sync.dma_start(selT, T["selT"])
            nc.sync.dma_start(selT16, T["selT16"].rearrange("g p c -> p g c"))
            nc.sync.dma_start(ones1, T["ones1"])
            nc.sync.dma_start(ones1r, T["ones1r"])
            nsk_box = [None]
            pending = []

            def epilogue(m, sqv):
                g = m % 8
                if m < 8:
                    # q: factor = 1/sqrt(sumsq), broadcast the pair's factor
                    # rows over 128 feat rows, normalize in place
                    nps = bcp.tile([2, 1024], F32, tag="bcnsk", name="nsq")
                    for hf in range(2):
                        nc.tensor.matmul(
                            nps[:, 512 * hf : 512 * hf + 256],
                            sel,
                            sqv[:, hf],
                            start=True,
                            stop=True,
                        )
                    npsv = nps.rearrange("p (b c) -> p b c", c=512)[:, :, 0:256]
                    rpair = sqpool.tile([2, 2, 256], F16, tag="rpair")
                    nc.scalar.activation(rpair, npsv, AF.Sqrt, bias=eps2)
                    nc.vector.reciprocal(rpair, rpair)
                    bc = bcp.tile([128, 1024], F32, tag="bcnsk", name="bc")
                    for hf in range(2):
                        nc.tensor.matmul(
                            bc[:, 512 * hf : 512 * hf + 256],
                            selT,
                            rpair[:, hf],
                            start=True,
                            stop=True,
                        )
                    bcv = bc.rearrange("p (b c) -> p b c", c=512)[:, :, 0:256]
                    dstv = qn_sb[:, g][:, 16:528].rearrange(
                        "p (b c) -> p b c", c=256
                    )
                    nc.vector.tensor_tensor(dstv, dstv, bcv, OP.mult)
                else:
                    # scatter-accumulate all 16 k sumsq rows into one tile;
                    # the single boundary sqrt keeps ACT off the sqrt table
                    # once attention's exps start
                    if nsk_box[0] is None:
                        nsk_box[0] = bcp.tile(
                            [16, 1024], F32, tag="bcnsk", name="nsk"
                        )
                    for hf in range(2):
                        nc.tensor.matmul(
                            nsk_box[0][:, 512 * hf : 512 * hf + 272],
                            sel16[:, g],
                            sqv[:, hf],
                            start=(g == 0),
                            stop=(g == 7),
                        )

            for mp in range(8):          # pair of feature tiles
                is_q = mp < 4
                # q features need only the 512 live query columns [16:528);
                # k features need all 544 (keys include the halo)
                w = 256 if is_q else 272
                x0 = 16 if is_q else 0
                pss = [
                    ps1.tile([128, 1024], F32, tag="qkps", name=f"qkps{i}")
                    for i in range(2)
                ]
                if mp in wq_prefetch:
                    wt = wq_prefetch.pop(mp)
                else:
                    wt = wpool.tile([128, KT, 256], F8, tag="wqk")
                    nc.sync.dma_start(
                        wt,
                        T["wqk"][:, 256 * mp : 256 * (mp + 1)].rearrange(
                            "(kt p) n -> p kt n", p=128
                        ),
                    )
                if not _do_p1:
                    continue
                for ktp in range(KT // 2):
                    for mi in range(2):
                        for hf in range(2):
                            nc.tensor.matmul(
                                pss[mi][:, 512 * hf : 512 * hf + w],
                                wt[:, 2 * ktp : 2 * ktp + 2, 128 * mi : 128 * (mi + 1)],
                                xT8[:, 2 * ktp : 2 * ktp + 2, x0 + w * hf : x0 + w * (hf + 1)],
                                start=(ktp == 0),
                                stop=(ktp == KT // 2 - 1),
                                perf_mode=mybir.MatmulPerfMode.DoubleRow,
                            )
                for mi in range(2):
                    m = 2 * mp + mi
                    g = m % 8
                    psv = pss[mi].rearrange("p (b c) -> p b c", c=512)[:, :, 0:w]
                    bias_col = qb2[:, m : m + 1]
                    dst = (qn_sb if m < 8 else k_sb)[:, g]
                    dstv = dst[:, x0 : x0 + 2 * w].rearrange(
                        "p (b c) -> p b c", c=w
                    )
                    nc.scalar.activation(dstv, psv, AF.Identity, bias=bias_col)
                    # squares on the idle Pool engine from the evicted fp16
                    # values (must read dst before the epilogue normalizes it)
                    sq = sqpool.tile([128, LOCAL], F16, tag="sq")
                    sqv = sq[:, 0 : 2 * w].rearrange("p (b c) -> p b c", c=w)
                    if os.environ.get("BIS_SQACT"):
                        nc.scalar.activation(
                            sqv, psv, bias=bias_col, func=AF.Square
                        )
                    else:
                        nc.gpsimd.tensor_tensor(sqv, dstv, dstv, OP.mult)
                    pending.append((m, sqv))
                while len(pending) > 2:
                    epilogue(*pending.pop(0))
            while pending:
                epilogue(*pending.pop(0))
            if _do_p1:
                nskv = nsk_box[0].rearrange("p (b c) -> p b c", c=512)[
                    :, :, 0:272
                ]
                rkv = rk.rearrange("p (b c) -> p b c", c=272)
                nc.scalar.activation(rkv, nskv, AF.Sqrt, bias=eps16)
                nc.vector.reciprocal(rk, rk)
                # normalize k in place, one head pair per feature tile
                rkv2 = rk.rearrange("p (b c) -> p b c", c=272)
                for g in range(8):
                    bck = bcp.tile([128, 1024], F32, tag="bcnsk", name="bck")
                    for hf in range(2):
                        nc.tensor.matmul(
                            bck[:, 512 * hf : 512 * hf + 272],
                            selT16[:, g],
                            rkv2[:, hf],
                            start=True,
                            stop=True,
                        )
                    bckv = bck.rearrange("p (b c) -> p b c", c=512)[:, :, 0:272]
                    dstv = k_sb[:, g].rearrange("p (b c) -> p b c", c=272)
                    nc.vector.tensor_tensor(dstv, dstv, bckv, OP.mult)

        # ---- phase 1c + 2 + 3: v-proj overlapped with attention; ------
        # ---- out-projection interleaved per query block ----------------
        wv = pp.tile([128, KT, D], F16)              # Wv^T (in-feat, v-feat)
        nc.scalar.dma_start(wv, T["wv"].rearrange("(kt p) n -> p kt n", p=128))

        nc.sync.dma_start(vb, T["vb"])
        nc.gpsimd.memset(v_h[:, :, :, 64:65], 1.0)
        nc.scalar.dma_start(masks, T["masks"])
        for i in range(3):
            nc.gpsimd.memset(_win(pabs[i][:, 160:], 512, 2, 192), 0.0)
            nc.gpsimd.memset(_win(pcs[i], 256, 2, 224), 0.0)
        nc.sync.dma_start(ob, T["ob"])
        ow = pp.tile([128, KT, D], F16)              # out_w^T (feat, out)
        nc.scalar.dma_start(ow, T["ow"].rearrange("(kt p) n -> p kt n", p=128))

        def v_tile(vps, m):
            M = 128 if m < 4 else LOCAL - 512
            for nh in range(2):
                vp = vps.tile([128, 512], F32, tag="vps", name="vp")
                for kt in range(KT):
                    nc.tensor.matmul(
                        vp[0:M],
                        xT[:, kt, 128 * m : 128 * m + M],
                        wv[:, kt, 512 * nh : 512 * (nh + 1)],
                        start=(kt == 0),
                        stop=False,
                    )
                nc.tensor.matmul(
                    vp[0:M],
                    ones1[:, 0:M],
                    vb[:, 512 * nh : 512 * (nh + 1)],
                    start=False,
                    stop=True,
                )
                vpv = vp[0:M].rearrange("p (h c) -> p h c", c=64)
                nc.vector.tensor_copy(
                    v_h[0:M, m, 8 * nh : 8 * (nh + 1), 0:64], vpv
                )

        it = 0
        ot2_box = [None]

        st_box = [None]

        def att(stp, otp, dnp, rrp, h, j):
            nonlocal it
            g, a = h // 2, h % 2
            q0 = 256 * j
            kx = k_sb[64 * a : 64 * a + 64]
            qx = qn_sb[64 * a : 64 * a + 64]
            # pair tiles: head a occupies columns [512a : 512a+512); the C
            # chunk's (32, 32) scores land in the head's dead zone [160:192)
            if a == 0:
                st_box[0] = stp.tile([128, 1024], F32, tag="st", name="st")
            st = st_box[0][:, 512 * a : 512 * (a + 1)]
            pab2 = pabs[(it // 2) % 3]
            pc2 = pcs[(it // 2) % 3]
            it += 1
            # score matmuls compute only each chunk's live query window:
            # A keys see q [0:160), B keys q [96:256), C keys q [224:256)
            nc.tensor.matmul(
                st[:, 0:160],
                kx[:, g, q0 : q0 + 128],
                qx[:, g, 16 + q0 : 16 + q0 + 160],
                start=True, stop=True,
            )
            nc.tensor.matmul(
                st[:, 352:512],
                kx[:, g, q0 + 128 : q0 + 256],
                qx[:, g, 16 + q0 + 96 : 16 + q0 + 256],
                start=True, stop=True,
            )
            nc.tensor.matmul(
                st[0:32, 160:192],
                kx[:, g, q0 + 256 : q0 + 288],
                qx[:, g, 16 + q0 + 224 : 16 + q0 + 256],
                start=True, stop=True,
            )
            if a == 0:
                ot2_box[0] = otp.tile([65, 512], F32, tag="ot", name="ot")
                return
            st2 = st_box[0]
            # exp straight from PSUM scores (ACT), then multiplicative 0/1
            # masking on the idle Pool engine - DVE stays out of the loop.
            # One exp + one mask-mult per head pair (4 windows, 4D AP); the
            # doubled mask tile supplies identical windows for both heads.
            m2 = masks[:, j].rearrange("p r c q -> p (r c q)")
            pabw = _win4(pab2, 512, 352, 160)
            nc.scalar.activation(pabw, _win4(st2, 512, 352, 160), AF.Exp)
            nc.gpsimd.tensor_tensor(
                pabw, pabw, _win4(m2, 768, 352, 160), OP.mult
            )
            pcw = _win(pc2[:, 224:], 256, 2, 32)[0:32]
            nc.scalar.activation(
                pcw, _win(st2[:, 160:], 512, 2, 32)[0:32], AF.Exp
            )
            nc.gpsimd.tensor_tensor(
                pcw, pcw, masks[0:32, j, :, 2, 224:256], OP.mult
            )
            for aa in range(2):
                pabh = pab2[:, 512 * aa : 512 * (aa + 1)]
                pch = pc2[:, 256 * aa : 256 * (aa + 1)]
                hh = 2 * g + aa
                ot = ot2_box[0][:, 256 * aa : 256 * (aa + 1)]
                # A streams all 256 q (seeding has_written); B and C
                # accumulate only their live query windows.
                nc.tensor.matmul(
                    ot, v_h[:, 2 * j, hh], pabh[:, 0:256],
                    start=True, stop=False,
                )
                nc.tensor.matmul(
                    ot[:, 96:256], v_h[:, 2 * j + 1, hh], pabh[:, 352:512],
                    start=False, stop=False,
                )
                nc.tensor.matmul(
                    ot[:, 224:256], v_h[0:32, 2 * j + 2, hh], pch[:, 224:256],
                    start=False, stop=True,
                )
            ot2 = ot2_box[0]
            rr = rrp.tile([1, 512], F16, tag="rr", name="rr")
            nc.vector.reciprocal(rr, ot2[64:65])
            dn = dnp.tile([64, 512], F32, tag="dn", name="dn")
            nc.tensor.matmul(dn, ones1r, rr, start=True, stop=True)
            otS = rrp.tile([64, 512], F32, tag="otS", name="otS")
            nc.scalar.activation(otS, ot2[0:64], AF.Copy)
            for aa in range(2):
                nc.vector.tensor_tensor(
                    outTn[64 * aa : 64 * aa + 64, g, q0 : q0 + 256],
                    otS[:, 256 * aa : 256 * (aa + 1)],
                    dn[:, 256 * aa : 256 * (aa + 1)],
                    OP.mult,
                )

        def outproj(yps, ysbp, qb):
            ysb = ysbp.tile([128, 1024], F16, tag="ysb", name="ysb")
            for nh in range(2):
                yp = yps.tile([128, 512], F32, tag="yps", name="yp")
                for kt in range(KT):
                    nc.tensor.matmul(
                        yp,
                        outTn[:, kt, 128 * qb : 128 * (qb + 1)],
                        ow[:, kt, 512 * nh : 512 * (nh + 1)],
                        start=(kt == 0),
                        stop=False,
                    )
                nc.tensor.matmul(
                    yp,
                    ones1,
                    ob[:, 512 * nh : 512 * (nh + 1)],
                    start=False,
                    stop=True,
                )
                nc.scalar.activation(
                    ysb[:, 512 * nh : 512 * (nh + 1)], yp, AF.Copy
                )
            nc.sync.dma_start(T["y"][128 * qb : 128 * (qb + 1)], ysb)

        # ablation probes for timing attribution (timing-only; breaks output)
        _ablate = int(os.environ.get("PROBE_ABLATE", "0"))
        do_outproj = _ablate < 1
        do_att = _ablate < 2
        do_vproj = _ablate < 3
        do_p1 = _ablate < 4

        with (
            tc.tile_pool(name="rr", bufs=6) as rrp,
            tc.tile_pool(name="ysb", bufs=3) as ysbp,
            tc.tile_pool(name="stp", bufs=2, space="PSUM") as stp,
            tc.tile_pool(name="otp", bufs=2, space="PSUM") as otp,
            tc.tile_pool(name="dnp", bufs=1, space="PSUM") as dnp,
        ):
            with tc.tile_pool(name="vps", bufs=1, space="PSUM") as vps:
                if do_vproj:
                    for m in range(3):
                        v_tile(vps, m)
                for h in range(H):
                    if do_vproj:
                        if h == 0:
                            v_tile(vps, 3)
                        if h == 2:
                            v_tile(vps, 4)
                    if do_att:
                        att(stp, otp, dnp, rrp, h, 0)
            with tc.tile_pool(name="yps", bufs=1, space="PSUM") as yps:
                if do_outproj:
                    outproj(yps, ysbp, 0)
                    outproj(yps, ysbp, 1)
                if do_att:
                    for h in range(H):
                        att(stp, otp, dnp, rrp, h, 1)
                if do_outproj:
                    outproj(yps, ysbp, 2)
                    outproj(yps, ysbp, 3)


_PROGRAM = None


def _declare(nc):
    T = {}

    def inp(name, shape, dt=FR):
        T[name] = nc.dram_tensor(name, shape, dt, kind="ExternalInput").ap()

    inp("xT", (D, LOCAL), F16)
    inp("xT8", (D, LOCAL), F8)
    inp("wqk", (D, 2 * D), F8)
    inp("wv", (D, D), F16)
    inp("ow", (D, D), F16)
    inp("masks", (128, 2, 2, 3, 256), F16)
    inp("qb2", (2 * D,), F32)
    inp("vb", (1, D), F16)
    inp("ob", (1, D), F16)
    inp("sel", (128, 2), F16)
    inp("sel16", (8, 128, 16), F16)
    inp("selT", (2, 128), F16)
    inp("selT16", (8, 16, 128), F16)
    inp("ones1", (1, 128), F16)
    inp("ones1r", (1, 64), F16)
    T["y"] = nc.dram_tensor("y", (CHUNK, D), F16, kind="ExternalOutput").ap()
    return T


def _build_program():
    global _PROGRAM
    if _PROGRAM is not None:
        return _PROGRAM
    nc = bacc.Bacc(
        "TRN2",
        target_bir_lowering=False,
        debug=False,
        enable_asserts=False,
        num_devices=NCORES,
    )
    T = _declare(nc)

    with tile.TileContext(nc) as tc:
        with nc.allow_low_precision(reason="fp16/fp32r matmul pipeline"):
            _emit(tc, T)
    nc.compile()
    _PROGRAM = nc
    return nc


def _host_masks(c0):
    """masks[p, j, rep, chunk, qq] for the core at chunk start c0; the
    mask is duplicated along rep so head-pair 4D APs can read it without
    stride-0 broadcast."""
    out = np.zeros((2, 3, 128, 256), dtype=np.float32)
    for j in range(2):
        qtok = c0 + 256 * j + np.arange(256)[None, :]          # (1, 256)
        for ci, (base, rows) in enumerate(((0, 128), (128, 128), (256, 32))):
            ktok = c0 - HALO + 256 * j + base + np.arange(rows)[:, None]  # (rows, 1)
            diff = ktok - qtok
            ok = (
                (np.abs(diff) <= KWIN * DIL)
                & (diff % DIL == 0)
                & (ktok >= 0)
                & (ktok < N)
            )
            out[j, ci, :rows][ok] = 1.0
    m = np.ascontiguousarray(out.transpose(2, 0, 1, 3)).astype(np.float16)
    return np.repeat(m[:, :, None], 2, axis=2)  # (128, 2, 2, 3, 256)


def _host_inputs(x, qkv_w, qkv_b, out_w, out_b):
    F8NP = ml_dtypes.float8_e4m3fn
    wqk = np.ascontiguousarray(qkv_w[: 2 * D].T.astype(F8NP))          # (D, 2D)
    wv = np.ascontiguousarray(qkv_w[2 * D :].T.astype(np.float16))     # (D, D)
    ow = np.ascontiguousarray(out_w.T.astype(np.float16))              # (D, D)
    qb2 = np.ascontiguousarray(qkv_b[: 2 * D])
    vb = np.ascontiguousarray(qkv_b[2 * D :].reshape(1, D).astype(np.float16))
    ob = np.ascontiguousarray(out_b.reshape(1, D).astype(np.float16))
    sel = np.zeros((128, 2), dtype=np.float16)
    sel[:64, 0] = 1.0
    sel[64:, 1] = 1.0
    selT = np.ascontiguousarray(sel.T.astype(np.float16))
    sel16 = np.zeros((8, 128, 16), dtype=np.float16)
    for g in range(8):
        sel16[g, :64, 2 * g] = 1.0
        sel16[g, 64:, 2 * g + 1] = 1.0
    selT16 = np.zeros((8, 16, 128), dtype=np.float16)
    for g in range(8):
        selT16[g, 2 * g, 0:64] = 1.0
        selT16[g, 2 * g + 1, 64:128] = 1.0
    ones1 = np.ones((1, 128), dtype=np.float16)
    ones1r = np.ones((1, 64), dtype=np.float16)

    in_maps = []
    for c in range(NCORES):
        b, i = divmod(c, 4)
        c0 = CHUNK * i
        xT = np.zeros((D, LOCAL), dtype=np.float16)
        lo, hi = max(0, c0 - HALO), min(N, c0 + CHUNK + HALO)
        xT[:, lo - (c0 - HALO) : hi - (c0 - HALO)] = x[b, lo:hi].T.astype(
            np.float16
        )
        in_maps.append(
            {
                "xT": xT,
                "xT8": xT.astype(F8NP),
                "wqk": wqk,
                "wv": wv,
                "ow": ow,
                "masks": _host_masks(c0),
                "qb2": qb2,
                "vb": vb,
                "ob": ob,
                "sel": sel,
                "sel16": sel16,
                "selT": selT,
                "selT16": selT16,
                "ones1": ones1,
                "ones1r": ones1r,
            }
        )
    return in_maps


def kernel(x, qkv_w, qkv_b, out_w, out_b):
    x = np.asarray(x, dtype=np.float32)
    qkv_w = np.asarray(qkv_w, dtype=np.float32)
    qkv_b = np.asarray(qkv_b, dtype=np.float32)
    out_w = np.asarray(out_w, dtype=np.float32)
    out_b = np.asarray(out_b, dtype=np.float32)

    nc = _build_program()
    in_maps = _host_inputs(x, qkv_w, qkv_b, out_w, out_b)
    res = bass_utils.run_bass_kernel_spmd(nc, in_maps, core_ids=list(range(NCORES)))

    out = np.empty((B, N, D), dtype=np.float32)
    for c in range(NCORES):
        b, i = divmod(c, 4)
        out[b, CHUNK * i : CHUNK * (i + 1)] = res.results[c]["y"].astype(
            np.float32
        )
    return out



# revision 65
# speedup vs baseline: 1.0830x; 1.0830x over previous
"""Dilated MHSA block on 8 Trainium2 NeuronCores.

Sharding: sequence-parallel. Core c (0..7) handles batch b=c//4, query chunk
[512*(c%4), 512*(c%4)+512) with a 16-token halo of keys/values on each side.
Each core computes its full 512x1024 output slice; the host just concatenates.

All heavy matmuls take fp16 inputs (1 cycle/row on the PE at any moving dim;
an fp16 input carries the same 11-bit significand the PE's tf32/fp32r mode
would keep from fp32) and accumulate in fp32 PSUM, so inputs ship as fp16 -
half the DMA traffic - at fp32r-equivalent accuracy (~5e-4 rel err).
Normalization scalars stay fp32/fp32r.

Per-core device pipeline:
  1a. q,k projection qkT = Wqk @ x^T in (feature, token) layout, one weight
      DMA per feature-tile pair; ACT evicts PSUM with per-feature bias
      (Identity) and squares (Square); per-pair norm reductions via selector
      matmuls are software-pipelined one tile behind the main matmuls.
  1b. QK-norm: ACT sqrt, DVE eps+reciprocal; q is normalized via a rank-2
      broadcast matmul + DVE multiply; k's factor is folded into the exp
      scale (per-partition in keys-on-partitions layout) after PE-transposing
      the (16, 544) factor block to (token, head) tiles.
  1c. v projection in natural (token, feat) layout with token tiles shifted
      -16 so banded attention reads aligned v tiles; heads stored at stride
      65 with a ones column so the AV matmul also emits the softmax
      denominator. Overlapped with the j=0 attention sweep (PE-heavy vs
      DVE/ACT-heavy) under a shared PSUM budget.
  2.  Per (head, 256-query block): 3 score matmuls restricted to each key
      chunk's live query window (keys 128/128/32 on partitions, 160/160/32
      q free), one merged DVE mask-add over the two live 160-wide windows
      (strided 3D AP) + a (32, 32) add for the tail chunk, ACT exp on live
      windows only (scale = 1/(|k|+eps)), 3 AV matmuls; adjacent head pairs
      pack their AV outputs into one (65, 512) PSUM bank so the DVE
      reciprocal, rank-1 broadcast matmul, and ACT evict run once per pair
      at 512 width, then per-head DVE multiplies -> normalized outT.
      P-tile dead regions are zeroed once via DMA and never rewritten.
  3.  Output projection y = outT^T @ out_w^T + b (bias as rank-1 K-append),
      interleaved per query block with the attention sweeps so the PE-bound
      projection hides under elementwise-bound attention.
"""

import os
import sys

for _p in ("/opt/trn_rl_repo", "/root/.axon_site/_ro/trn_rl_repo"):
    if os.path.isdir(_p) and _p not in sys.path:
        sys.path.insert(0, _p)

import ml_dtypes
import numpy as np

import concourse.bass as bass
import concourse.mybir as mybir
import concourse.tile as tile
from concourse import bacc
from concourse import bass_utils

F32 = mybir.dt.float32
FR = mybir.dt.float32r  # fp32 bits, tf32 matmul mode (full rate at N >= 256)
F16 = mybir.dt.float16
F8 = mybir.dt.float8e4  # e4m3; q/k projection runs fp8 DoubleRow (2x PE)

B, N, D = 2, 2048, 1024
H, DH = 16, 64
KWIN, DIL = 8, 2
EPS = 1e-6
NCORES = 8
CHUNK = 512          # queries per core
HALO = 16            # KWIN * DIL
LOCAL = CHUNK + 2 * HALO  # 544 tokens (keys/values) per core
NEG = -30000.0       # mask value (exp(NEG * scale) == 0 for any sane scale)
KT = D // 128        # 8 contraction tiles


def _win(ap2d, stride, count, width):
    """(P, count, width) windowed view over a 2D AP's free dim."""
    return bass.AP(
        tensor=ap2d.tensor,
        offset=ap2d.offset,
        ap=[list(ap2d.ap[0]), [stride, count], [1, width]],
    )


def _win4(ap2d, s2, s1, width):
    """(P, 2, 2, width) doubly-windowed view over a 2D AP's free dim."""
    return bass.AP(
        tensor=ap2d.tensor,
        offset=ap2d.offset,
        ap=[list(ap2d.ap[0]), [s2, 2], [s1, 2], [1, width]],
    )


def _emit(tc, T):
    nc = tc.nc
    AF = mybir.ActivationFunctionType
    OP = mybir.AluOpType

    with tc.tile_pool(name="persist", bufs=1) as pp:
        # ---- early loads -------------------------------------------------
        xT = pp.tile([128, KT, LOCAL], F16)          # x^T, (in-feat, token)
        for kt in range(KT):
            nc.sync.dma_start(xT[:, kt], T["xT"][128 * kt : 128 * (kt + 1)])
        xT8 = pp.tile([128, KT, LOCAL], F8)          # fp8 copy for q/k proj
        nc.sync.dma_start(xT8, T["xT8"].rearrange("(kt p) n -> p kt n", p=128))

        wq_prefetch = {}

        qb2 = pp.tile([128, 16], F32)                # qkv bias for q,k
        sel = pp.tile([128, 2], F16)                  # head-pair sum selector
        sel16 = pp.tile([128, 8, 16], F16)            # k-side scatter selectors
        selT = pp.tile([2, 128], F16)                # head-pair bcast selector
        selT16 = pp.tile([16, 8, 128], F16)          # k-pair bcast selectors
        ones1 = pp.tile([1, 128], F16)
        ones1r = pp.tile([1, 64], F16)
        eps2 = pp.tile([2, 1], F32)
        nc.gpsimd.memset(eps2, 1e-8)
        eps16 = pp.tile([16, 1], F32)
        nc.gpsimd.memset(eps16, 1e-8)
        rk = pp.tile([16, LOCAL], F16)               # 1/|k| per (head, token)

        v_sb = pp.tile([128, 5, H * 65], F16)         # v natural, 65-stride heads
        v_h = v_sb.rearrange("p m (h c) -> p m h c", c=65)

        pabs = [pp.tile([128, 1024], F16, name=f"pab{i}") for i in range(3)]
        pcs = [pp.tile([32, 512], F16, name=f"pc{i}") for i in range(3)]
        masks = pp.tile([128, 2, 2, 3, 256], F16)    # (p, j, rep, chunk, q)
        vb = pp.tile([1, D], F16)
        ob = pp.tile([1, D], F16)

        k_sb = pp.tile([128, 8, LOCAL], F16)          # k^T normalized, 2 heads/tile
        qn_sb = pp.tile([128, 8, LOCAL], F16)         # q^T normalized
        outTn = pp.tile([128, KT, CHUNK], F16)        # attn out^T (feat, q)

        # ---- phase 1a/1b: q,k projection + QK norm ----------------------
        _do_p1 = int(os.environ.get("PROBE_ABLATE", "0")) < 4
        with (
            tc.tile_pool(name="wpool", bufs=8) as wpool,
            tc.tile_pool(name="sqpool", bufs=6) as sqpool,
            tc.tile_pool(name="ps1", bufs=2, space="PSUM") as ps1,
            tc.tile_pool(name="bcp", bufs=2, space="PSUM") as bcp,
        ):
            for mp in range(2):
                wt = wpool.tile([128, KT, 256], F8, tag="wqk", name=f"wtp{mp}")
                nc.sync.dma_start(
                    wt,
                    T["wqk"][:, 256 * mp : 256 * (mp + 1)].rearrange(
                        "(kt p) n -> p kt n", p=128
                    ),
                )
                wq_prefetch[mp] = wt
            nc.sync.dma_start(qb2, T["qb2"].rearrange("(m p) -> p m", p=128))
            nc.sync.dma_start(sel, T["sel"])
            nc.sync.dma_start(sel16, T["sel16"].rearrange("g p c -> p g c"))
            nc.sync.dma_start(selT, T["selT"])
            nc.sync.dma_start(selT16, T["selT16"].rearrange("g p c -> p g c"))
            nc.sync.dma_start(ones1, T["ones1"])
            nc.sync.dma_start(ones1r, T["ones1r"])
            nsk_box = [None]
            pending = []

            def epilogue(m, sqv):
                g = m % 8
                if m < 8:
                    # q: factor = 1/sqrt(sumsq), broadcast the pair's factor
                    # rows over 128 feat rows, normalize in place
                    nps = bcp.tile([2, 1024], F32, tag="bcnsk", name="nsq")
                    sqf = sqv.rearrange("p b c -> p (b c)")
                    nc.tensor.matmul(
                        nps[:, 0:512], sel, sqf, start=True, stop=True
                    )
                    npsv = nps.rearrange("p (b c) -> p b c", c=512)[:, :, 0:256]
                    rpair = sqpool.tile([2, 2, 256], F16, tag="rpair")
                    nc.scalar.activation(
                        rpair, nps[:, 0:512].rearrange("p (b c) -> p b c", c=256),
                        AF.Sqrt, bias=eps2,
                    )
                    nc.vector.reciprocal(rpair, rpair)
                    bc = bcp.tile([128, 1024], F32, tag="bcnsk", name="bc")
                    nc.tensor.matmul(
                        bc[:, 0:512],
                        selT,
                        rpair.rearrange("p b c -> p (b c)"),
                        start=True,
                        stop=True,
                    )
                    bcv = bc[:, 0:512].rearrange("p (b c) -> p b c", c=256)
                    dstv = qn_sb[:, g][:, 16:528].rearrange(
                        "p (b c) -> p b c", c=256
                    )
                    nc.vector.tensor_tensor(dstv, dstv, bcv, OP.mult)
                else:
                    # scatter-accumulate k sumsq rows, two half-groups; each
                    # half's sqrt+recip+normalize runs as soon as it is done
                    # (both sqrts still precede every attention exp)
                    if nsk_box[0] is None:
                        nsk_box[0] = bcp.tile(
                            [16, 1024], F32, tag="bcnsk", name="nsk"
                        )
                    for hf in range(2):
                        nc.tensor.matmul(
                            nsk_box[0][:, 512 * hf : 512 * hf + 272],
                            sel16[:, g],
                            sqv[:, hf],
                            start=(g == 0),
                            stop=(g == 7),
                        )
                    if g == 7:
                        nc.scalar.activation(
                            rk.rearrange("p (b c) -> p b c", c=272),
                            nsk_box[0].rearrange("p (b c) -> p b c", c=512)[
                                :, :, 0:272
                            ],
                            AF.Sqrt,
                            bias=eps16,
                        )
                        nc.vector.reciprocal(rk, rk)
                        rkv2 = rk.rearrange("p (b c) -> p b c", c=272)
                        for gg in range(8):
                            bck = bcp.tile(
                                [128, 1024], F32, tag="bcnsk", name="bck"
                            )
                            for hf in range(2):
                                nc.tensor.matmul(
                                    bck[:, 512 * hf : 512 * hf + 272],
                                    selT16[:, gg],
                                    rkv2[:, hf],
                                    start=True,
                                    stop=True,
                                )
                            bckv = bck.rearrange("p (b c) -> p b c", c=512)[
                                :, :, 0:272
                            ]
                            dstv = k_sb[:, gg].rearrange(
                                "p (b c) -> p b c", c=272
                            )
                            nc.vector.tensor_tensor(dstv, dstv, bckv, OP.mult)

            for mp in range(8):          # pair of feature tiles
                is_q = mp < 4
                # q features need only the 512 live query columns [16:528);
                # k features need all 544 (keys include the halo)
                w = 256 if is_q else 272
                x0 = 16 if is_q else 0
                pss = [
                    ps1.tile([128, 1024], F32, tag="qkps", name=f"qkps{i}")
                    for i in range(2)
                ]
                if mp in wq_prefetch:
                    wt = wq_prefetch.pop(mp)
                else:
                    wt = wpool.tile([128, KT, 256], F8, tag="wqk")
                    nc.sync.dma_start(
                        wt,
                        T["wqk"][:, 256 * mp : 256 * (mp + 1)].rearrange(
                            "(kt p) n -> p kt n", p=128
                        ),
                    )
                if not _do_p1:
                    continue
                for ktp in range(KT // 2):
                    for mi in range(2):
                        for hf in range(2):
                            nc.tensor.matmul(
                                pss[mi][:, 512 * hf : 512 * hf + w],
                                wt[:, 2 * ktp : 2 * ktp + 2, 128 * mi : 128 * (mi + 1)],
                                xT8[:, 2 * ktp : 2 * ktp + 2, x0 + w * hf : x0 + w * (hf + 1)],
                                start=(ktp == 0),
                                stop=(ktp == KT // 2 - 1),
                                perf_mode=mybir.MatmulPerfMode.DoubleRow,
                            )
                for mi in range(2):
                    m = 2 * mp + mi
                    g = m % 8
                    psv = pss[mi].rearrange("p (b c) -> p b c", c=512)[:, :, 0:w]
                    bias_col = qb2[:, m : m + 1]
                    dst = (qn_sb if m < 8 else k_sb)[:, g]
                    dstv = dst[:, x0 : x0 + 2 * w].rearrange(
                        "p (b c) -> p b c", c=w
                    )
                    nc.scalar.activation(dstv, psv, AF.Identity, bias=bias_col)
                    # squares on the idle Pool engine from the evicted fp16
                    # values (must read dst before the epilogue normalizes it)
                    sq = sqpool.tile([128, LOCAL], F16, tag="sq")
                    sqv = sq[:, 0 : 2 * w].rearrange("p (b c) -> p b c", c=w)
                    if os.environ.get("BIS_SQACT"):
                        nc.scalar.activation(
                            sqv, psv, bias=bias_col, func=AF.Square
                        )
                    else:
                        nc.gpsimd.tensor_tensor(sqv, dstv, dstv, OP.mult)
                    pending.append((m, sqv))
                while len(pending) > 2:
                    epilogue(*pending.pop(0))
            while pending:
                epilogue(*pending.pop(0))

        # ---- phase 1c + 2 + 3: v-proj overlapped with attention; ------
        # ---- out-projection interleaved per query block ----------------
        wv = pp.tile([128, KT, D], F16)              # Wv^T (in-feat, v-feat)
        nc.sync.dma_start(wv, T["wv"].rearrange("(kt p) n -> p kt n", p=128))

        nc.sync.dma_start(vb, T["vb"])
        nc.gpsimd.memset(v_h[:, :, :, 64:65], 1.0)
        nc.sync.dma_start(masks, T["masks"])
        for i in range(3):
            nc.gpsimd.memset(_win(pabs[i][:, 160:], 512, 2, 192), 0.0)
            nc.gpsimd.memset(_win(pcs[i], 256, 2, 224), 0.0)
        nc.sync.dma_start(ob, T["ob"])
        ow = pp.tile([128, KT, D], F16)              # out_w^T (feat, out)
        nc.sync.dma_start(ow, T["ow"].rearrange("(kt p) n -> p kt n", p=128))

        def v_tile(vps, m):
            M = 128 if m < 4 else LOCAL - 512
            for nh in range(2):
                vp = vps.tile([128, 512], F32, tag="vps", name="vp")
                for kt in range(KT):
                    nc.tensor.matmul(
                        vp[0:M],
                        xT[:, kt, 128 * m : 128 * m + M],
                        wv[:, kt, 512 * nh : 512 * (nh + 1)],
                        start=(kt == 0),
                        stop=False,
                    )
                nc.tensor.matmul(
                    vp[0:M],
                    ones1[:, 0:M],
                    vb[:, 512 * nh : 512 * (nh + 1)],
                    start=False,
                    stop=True,
                )
                vpv = vp[0:M].rearrange("p (h c) -> p h c", c=64)
                if nh == 0:
                    nc.vector.tensor_copy(
                        v_h[0:M, m, 8 * nh : 8 * (nh + 1), 0:64], vpv
                    )
                else:
                    nc.scalar.activation(
                        v_h[0:M, m, 8 * nh : 8 * (nh + 1), 0:64], vpv, AF.Copy
                    )

        it = 0
        ot2_box = [None]

        st_box = [None]

        def att(stp, otp, dnp, rrp, h, j):
            nonlocal it
            g, a = h // 2, h % 2
            q0 = 256 * j
            kx = k_sb[64 * a : 64 * a + 64]
            qx = qn_sb[64 * a : 64 * a + 64]
            # pair tiles: head a occupies columns [512a : 512a+512); the C
            # chunk's (32, 32) scores land in the head's dead zone [160:192)
            if a == 0:
                st_box[0] = stp.tile([128, 1024], F32, tag="st", name="st")
            st = st_box[0][:, 512 * a : 512 * (a + 1)]
            pab2 = pabs[(it // 2) % 3]
            pc2 = pcs[(it // 2) % 3]
            it += 1
            # score matmuls compute only each chunk's live query window:
            # A keys see q [0:160), B keys q [96:256), C keys q [224:256)
            nc.tensor.matmul(
                st[:, 0:160],
                kx[:, g, q0 : q0 + 128],
                qx[:, g, 16 + q0 : 16 + q0 + 160],
                start=True, stop=True,
            )
            nc.tensor.matmul(
                st[:, 352:512],
                kx[:, g, q0 + 128 : q0 + 256],
                qx[:, g, 16 + q0 + 96 : 16 + q0 + 256],
                start=True, stop=True,
            )
            nc.tensor.matmul(
                st[0:32, 160:192],
                kx[:, g, q0 + 256 : q0 + 288],
                qx[:, g, 16 + q0 + 224 : 16 + q0 + 256],
                start=True, stop=True,
            )
            if a == 0:
                ot2_box[0] = otp.tile([65, 512], F32, tag="ot", name="ot")
                return
            st2 = st_box[0]
            # exp straight from PSUM scores (ACT), then multiplicative 0/1
            # masking on the idle Pool engine - DVE stays out of the loop.
            # One exp + one mask-mult per head pair (4 windows, 4D AP); the
            # doubled mask tile supplies identical windows for both heads.
            m2 = masks[:, j].rearrange("p r c q -> p (r c q)")
            pabw = _win4(pab2, 512, 352, 160)
            nc.scalar.activation(pabw, _win4(st2, 512, 352, 160), AF.Exp)
            nc.gpsimd.tensor_tensor(
                pabw, pabw, _win4(m2, 768, 352, 160), OP.mult
            )
            pcw = _win(pc2[:, 224:], 256, 2, 32)[0:32]
            nc.scalar.activation(
                pcw, _win(st2[:, 160:], 512, 2, 32)[0:32], AF.Exp
            )
            nc.gpsimd.tensor_tensor(
                pcw, pcw, masks[0:32, j, :, 2, 224:256], OP.mult
            )
            for aa in range(2):
                pabh = pab2[:, 512 * aa : 512 * (aa + 1)]
                pch = pc2[:, 256 * aa : 256 * (aa + 1)]
                hh = 2 * g + aa
                ot = ot2_box[0][:, 256 * aa : 256 * (aa + 1)]
                # A streams all 256 q (seeding has_written); B and C
                # accumulate only their live query windows.
                nc.tensor.matmul(
                    ot, v_h[:, 2 * j, hh], pabh[:, 0:256],
                    start=True, stop=False,
                )
                nc.tensor.matmul(
                    ot[:, 96:256], v_h[:, 2 * j + 1, hh], pabh[:, 352:512],
                    start=False, stop=False,
                )
                nc.tensor.matmul(
                    ot[:, 224:256], v_h[0:32, 2 * j + 2, hh], pch[:, 224:256],
                    start=False, stop=True,
                )
            ot2 = ot2_box[0]
            rr = rrp.tile([1, 512], F16, tag="rr", name="rr")
            nc.vector.reciprocal(rr, ot2[64:65])
            dn = dnp.tile([64, 512], F32, tag="dn", name="dn")
            nc.tensor.matmul(dn, ones1r, rr, start=True, stop=True)
            otS = rrp.tile([64, 512], F32, tag="otS", name="otS")
            nc.scalar.activation(otS, ot2[0:64], AF.Copy)
            for aa in range(2):
                nc.vector.tensor_tensor(
                    outTn[64 * aa : 64 * aa + 64, g, q0 : q0 + 256],
                    otS[:, 256 * aa : 256 * (aa + 1)],
                    dn[:, 256 * aa : 256 * (aa + 1)],
                    OP.mult,
                )

        def outproj(yps, ysbp, qb):
            ysb = ysbp.tile([128, 1024], F16, tag="ysb", name="ysb")
            for nh in range(2):
                yp = yps.tile([128, 512], F32, tag="yps", name="yp")
                for kt in range(KT):
                    nc.tensor.matmul(
                        yp,
                        outTn[:, kt, 128 * qb : 128 * (qb + 1)],
                        ow[:, kt, 512 * nh : 512 * (nh + 1)],
                        start=(kt == 0),
                        stop=False,
                    )
                nc.tensor.matmul(
                    yp,
                    ones1,
                    ob[:, 512 * nh : 512 * (nh + 1)],
                    start=False,
                    stop=True,
                )
                nc.scalar.activation(
                    ysb[:, 512 * nh : 512 * (nh + 1)], yp, AF.Copy
                )
            nc.sync.dma_start(T["y"][128 * qb : 128 * (qb + 1)], ysb)

        # ablation probes for timing attribution (timing-only; breaks output)
        _ablate = int(os.environ.get("PROBE_ABLATE", "0"))
        do_outproj = _ablate < 1
        do_att = _ablate < 2
        do_vproj = _ablate < 3
        do_p1 = _ablate < 4

        with (
            tc.tile_pool(name="rr", bufs=6) as rrp,
            tc.tile_pool(name="ysb", bufs=3) as ysbp,
            tc.tile_pool(name="stp", bufs=2, space="PSUM") as stp,
            tc.tile_pool(name="otp", bufs=2, space="PSUM") as otp,
            tc.tile_pool(name="dnp", bufs=1, space="PSUM") as dnp,
        ):
            with tc.tile_pool(name="vps", bufs=1, space="PSUM") as vps:
                if do_vproj:
                    for m in range(3):
                        v_tile(vps, m)
                for h in range(H):
                    if do_vproj:
                        if h == 0:
                            v_tile(vps, 3)
                        if h == 2:
                            v_tile(vps, 4)
                    if do_att:
                        att(stp, otp, dnp, rrp, h, 0)
            with tc.tile_pool(name="yps", bufs=1, space="PSUM") as yps:
                if do_att:
                    for h in range(H):
                        if do_outproj and h == 1:
                            outproj(yps, ysbp, 0)
                        if do_outproj and h == 6:
                            outproj(yps, ysbp, 1)
                        att(stp, otp, dnp, rrp, h, 1)
                elif do_outproj:
                    outproj(yps, ysbp, 0)
                    outproj(yps, ysbp, 1)
                if do_outproj:
                    outproj(yps, ysbp, 2)
                    outproj(yps, ysbp, 3)


_PROGRAM = None


def _declare(nc):
    T = {}

    def inp(name, shape, dt=FR):
        T[name] = nc.dram_tensor(name, shape, dt, kind="ExternalInput").ap()

    inp("xT", (D, LOCAL), F16)
    inp("xT8", (D, LOCAL), F8)
    inp("wqk", (D, 2 * D), F8)
    inp("wv", (D, D), F16)
    inp("ow", (D, D), F16)
    inp("masks", (128, 2, 2, 3, 256), F16)
    inp("qb2", (2 * D,), F32)
    inp("vb", (1, D), F16)
    inp("ob", (1, D), F16)
    inp("sel", (128, 2), F16)
    inp("sel16", (8, 128, 16), F16)
    inp("selT", (2, 128), F16)
    inp("selT16", (8, 16, 128), F16)
    inp("ones1", (1, 128), F16)
    inp("ones1r", (1, 64), F16)
    T["y"] = nc.dram_tensor("y", (CHUNK, D), F16, kind="ExternalOutput").ap()
    return T


def _build_program():
    global _PROGRAM
    if _PROGRAM is not None:
        return _PROGRAM
    nc = bacc.Bacc(
        "TRN2",
        target_bir_lowering=False,
        debug=False,
        enable_asserts=False,
        num_devices=NCORES,
    )
    T = _declare(nc)

    with tile.TileContext(nc) as tc:
        with nc.allow_low_precision(reason="fp16/fp32r matmul pipeline"):
            _emit(tc, T)
    nc.compile()
    _PROGRAM = nc
    return nc


def _host_masks(c0):
    """masks[p, j, rep, chunk, qq] for the core at chunk start c0; the
    mask is duplicated along rep so head-pair 4D APs can read it without
    stride-0 broadcast."""
    out = np.zeros((2, 3, 128, 256), dtype=np.float32)
    for j in range(2):
        qtok = c0 + 256 * j + np.arange(256)[None, :]          # (1, 256)
        for ci, (base, rows) in enumerate(((0, 128), (128, 128), (256, 32))):
            ktok = c0 - HALO + 256 * j + base + np.arange(rows)[:, None]  # (rows, 1)
            diff = ktok - qtok
            ok = (
                (np.abs(diff) <= KWIN * DIL)
                & (diff % DIL == 0)
                & (ktok >= 0)
                & (ktok < N)
            )
            out[j, ci, :rows][ok] = 1.0
    m = np.ascontiguousarray(out.transpose(2, 0, 1, 3)).astype(np.float16)
    return np.repeat(m[:, :, None], 2, axis=2)  # (128, 2, 2, 3, 256)


def _host_inputs(x, qkv_w, qkv_b, out_w, out_b):
    F8NP = ml_dtypes.float8_e4m3fn
    wqk = np.ascontiguousarray(qkv_w[: 2 * D].T.astype(F8NP))          # (D, 2D)
    wv = np.ascontiguousarray(qkv_w[2 * D :].T.astype(np.float16))     # (D, D)
    ow = np.ascontiguousarray(out_w.T.astype(np.float16))              # (D, D)
    qb2 = np.ascontiguousarray(qkv_b[: 2 * D])
    vb = np.ascontiguousarray(qkv_b[2 * D :].reshape(1, D).astype(np.float16))
    ob = np.ascontiguousarray(out_b.reshape(1, D).astype(np.float16))
    sel = np.zeros((128, 2), dtype=np.float16)
    sel[:64, 0] = 1.0
    sel[64:, 1] = 1.0
    selT = np.ascontiguousarray(sel.T.astype(np.float16))
    sel16 = np.zeros((8, 128, 16), dtype=np.float16)
    for g in range(8):
        sel16[g, :64, 2 * g] = 1.0
        sel16[g, 64:, 2 * g + 1] = 1.0
    selT16 = np.zeros((8, 16, 128), dtype=np.float16)
    for g in range(8):
        selT16[g, 2 * g, 0:64] = 1.0
        selT16[g, 2 * g + 1, 64:128] = 1.0
    ones1 = np.ones((1, 128), dtype=np.float16)
    ones1r = np.ones((1, 64), dtype=np.float16)

    in_maps = []
    for c in range(NCORES):
        b, i = divmod(c, 4)
        c0 = CHUNK * i
        xT = np.zeros((D, LOCAL), dtype=np.float16)
        lo, hi = max(0, c0 - HALO), min(N, c0 + CHUNK + HALO)
        xT[:, lo - (c0 - HALO) : hi - (c0 - HALO)] = x[b, lo:hi].T.astype(
            np.float16
        )
        in_maps.append(
            {
                "xT": xT,
                "xT8": xT.astype(F8NP),
                "wqk": wqk,
                "wv": wv,
                "ow": ow,
                "masks": _host_masks(c0),
                "qb2": qb2,
                "vb": vb,
                "ob": ob,
                "sel": sel,
                "sel16": sel16,
                "selT": selT,
                "selT16": selT16,
                "ones1": ones1,
                "ones1r": ones1r,
            }
        )
    return in_maps


def kernel(x, qkv_w, qkv_b, out_w, out_b):
    x = np.asarray(x, dtype=np.float32)
    qkv_w = np.asarray(qkv_w, dtype=np.float32)
    qkv_b = np.asarray(qkv_b, dtype=np.float32)
    out_w = np.asarray(out_w, dtype=np.float32)
    out_b = np.asarray(out_b, dtype=np.float32)

    nc = _build_program()
    in_maps = _host_inputs(x, qkv_w, qkv_b, out_w, out_b)
    res = bass_utils.run_bass_kernel_spmd(nc, in_maps, core_ids=list(range(NCORES)))

    out = np.empty((B, N, D), dtype=np.float32)
    for c in range(NCORES):
        b, i = divmod(c, 4)
        out[b, CHUNK * i : CHUNK * (i + 1)] = res.results[c]["y"].astype(
            np.float32
        )
    return out

